# revision 8
# baseline (speedup 1.0000x reference)
"""DynamicSparseRetriever Trainium2 kernel.

Batch-parallel over 8 NeuronCores: core b handles batch row b.
Per core:
  - project contexts through Wc on the PE (fp32), streaming 64 tiles of 512
    contexts from HBM in a feature-major layout prepared on the host
  - per-context u = q_hat . c_red and n2 = ||c_red||^2 via small stationary
    matmuls (outputs land ctx-on-partitions, ideal for top-k)
  - scores = u * rsqrt(n2) (Newton-refined), masked to -inf
  - token budget from the query-complexity MLP, computed on device
  - exact variable-k threshold via 34-step bisection on the score value
    (count(s >= t) via fused DVE compare+accumulate and a PE ones-reduction)
  - selection_mask = (s >= threshold)
Host only reshapes/shards inputs and inverts the layout on the way out.
"""
import os
import sys

for _p in ("/opt/trn_rl_repo", "/root/.axon_site/_ro/trn_rl_repo"):
    if os.path.isdir(_p) and _p not in sys.path:
        sys.path.insert(0, _p)

import numpy as np
import bass_rust
import concourse.bass as bass
import concourse.mybir as mybir
from concourse.tile import TileContext
from concourse.bass_utils import run_bass_kernel_spmd

F32 = mybir.dt.float32
F16 = mybir.dt.float16
U8 = mybir.dt.uint8
AF = mybir.ActivationFunctionType
ALU = mybir.AluOpType

B, Q, C, E, R, H = 8, 64, 32768, 1024, 128, 128
KCH = E // 128          # 8 feature chunks
T = 512                 # contexts per tile
NT = C // T             # 64 tiles
NCOL = C // 128         # 256 score columns
BISECT_ITERS = 30

_NC_CACHE = {}


def build_nc(C=C, bisect_iters=BISECT_ITERS):
    NT = C // T
    NCOL = C // 128
    nc = bass.Bass()
    xhi = nc.dram_tensor("xhi", [128, KCH, C], F16, kind="ExternalInput")
    xlo = nc.dram_tensor("xlo", [128, KCH, C], F16, kind="ExternalInput")
    q_nat = nc.dram_tensor("q_nat", [Q, E], F32, kind="ExternalInput")
    q_embT = nc.dram_tensor("q_embT", [128, KCH, Q], F32, kind="ExternalInput")
    wq = nc.dram_tensor("wq", [128, KCH, R], F32, kind="ExternalInput")
    wch = nc.dram_tensor("wch", [128, KCH, R], F16, kind="ExternalInput")
    wcl = nc.dram_tensor("wcl", [128, KCH, R], F16, kind="ExternalInput")
    w1 = nc.dram_tensor("w1", [128, KCH, H], F32, kind="ExternalInput")
    smalls = nc.dram_tensor("smalls", [128, 8], F32, kind="ExternalInput")
    mask_in = nc.dram_tensor("mask_in", [128, NCOL], U8, kind="ExternalInput")

    scores_out = nc.dram_tensor("scores_out", [128, NCOL], F32, kind="ExternalOutput")
    mask_out = nc.dram_tensor("mask_out", [128, NCOL], F32, kind="ExternalOutput")

    with TileContext(nc) as tc:
        with tc.tile_pool(name="const", bufs=1) as cp, \
             tc.tile_pool(name="xts", bufs=3) as xp, \
             tc.tile_pool(name="zs", bufs=2) as zp, \
             tc.tile_pool(name="psz", bufs=2, space="PSUM") as psz, \
             tc.tile_pool(name="psnu", bufs=3, space="PSUM") as psnu, \
             tc.tile_pool(name="psaux", bufs=2, space="PSUM") as psaux:

            # ---- constants ----
            wq_sb = cp.tile([128, KCH, R], F32)
            wch_sb = cp.tile([128, KCH, R], F16)
            wcl_sb = cp.tile([128, KCH, R], F16)
            w1_sb = cp.tile([128, KCH, H], F32)
            sm_sb = cp.tile([128, 8], F32)
            qn_sb = cp.tile([Q, E], F32)
            qt_sb = cp.tile([128, KCH, Q], F32)
            mask_sb = cp.tile([128, NCOL], U8)
            nc.sync.dma_start(out=wch_sb, in_=wch[:])
            nc.sync.dma_start(out=wcl_sb, in_=wcl[:])
            nc.sync.dma_start(out=wq_sb, in_=wq[:])
            nc.sync.dma_start(out=w1_sb, in_=w1[:])
            nc.sync.dma_start(out=sm_sb, in_=smalls[:])
            nc.sync.dma_start(out=qn_sb, in_=q_nat[:])
            nc.sync.dma_start(out=qt_sb, in_=q_embT[:])
            nc.sync.dma_start(out=mask_sb, in_=mask_in[:])

            ones128 = cp.tile([128, 1], F32)
            ones_row = cp.tile([1, 128], F32)
            ones64 = cp.tile([Q, 1], F32)
            zero_col = cp.tile([128, 1], F32)
            nc.vector.memset(ones128, 1.0)
            nc.vector.memset(ones_row, 1.0)
            nc.vector.memset(ones64, 1.0)
            nc.vector.memset(zero_col, 0.0)

            bq_ap = sm_sb[:, 1:2]
            bc_ap = sm_sb[:, 2:3]
            b1_ap = sm_sb[:, 3:4]
            w2_ap = sm_sb[:, 0:1]
            b2_ap = sm_sb[0:1, 4:5]

            # ---- prelude: q_hat = l2norm(mean(l2norm(q @ Wq + bq))) ----
            ps_qr = psaux.tile([128, Q], F32, tag="aux")
            for k in range(KCH):
                nc.tensor.matmul(ps_qr, wq_sb[:, k, :], qt_sb[:, k, :],
                                 start=(k == 0), stop=(k == KCH - 1))
            qred = cp.tile([128, Q], F32)
            nc.scalar.activation(qred, ps_qr, AF.Identity, bias=bq_ap)
            q2 = cp.tile([128, Q], F32)
            nc.vector.tensor_tensor(q2, qred, qred, op=ALU.mult)
            ps_n = psaux.tile([1, Q], F32, tag="aux")
            nc.tensor.matmul(ps_n, ones128, q2, start=True, stop=True)
            sqn = cp.tile([1, Q], F32)
            nc.scalar.activation(sqn, ps_n, AF.Sqrt, bias=zero_col[0:1, :])
            invn = cp.tile([1, Q], F32)
            nc.vector.reciprocal(invn, sqn)
            # one Newton step for 1/sqrt: y' = y * (1.5 - 0.5 * n2 * y^2)
            t_a = cp.tile([1, Q], F32, tag="nt_a")
            nc.vector.tensor_tensor(t_a, invn, invn, op=ALU.mult)
            nc.vector.tensor_tensor(t_a, t_a, ps_n, op=ALU.mult)
            nc.vector.tensor_scalar(t_a, t_a, -0.5, 1.5, op0=ALU.mult, op1=ALU.add)
            nc.vector.tensor_tensor(invn, invn, t_a, op=ALU.mult)
            ps_bc = psaux.tile([128, Q], F32, tag="aux")
            nc.tensor.matmul(ps_bc, ones_row, invn, start=True, stop=True)
            qu = cp.tile([128, Q], F32)
            nc.vector.tensor_tensor(qu, qred, ps_bc, op=ALU.mult)
            qp = cp.tile([128, 1], F32)
            nc.vector.reduce_sum(qp, qu, axis=mybir.AxisListType.X)
            qp2 = cp.tile([128, 1], F32)
            nc.vector.tensor_tensor(qp2, qp, qp, op=ALU.mult)
            ps_np = psaux.tile([1, 1], F32, tag="aux")
            nc.tensor.matmul(ps_np, ones128, qp2, start=True, stop=True)
            sqp = cp.tile([1, 1], F32)
            nc.scalar.activation(sqp, ps_np, AF.Sqrt, bias=zero_col[0:1, :])
            invp = cp.tile([1, 1], F32)
            nc.vector.reciprocal(invp, sqp)
            t_b = cp.tile([1, 1], F32, tag="nt_b")
            nc.vector.tensor_tensor(t_b, invp, invp, op=ALU.mult)
            nc.vector.tensor_tensor(t_b, t_b, ps_np, op=ALU.mult)
            nc.vector.tensor_scalar(t_b, t_b, -0.5, 1.5, op0=ALU.mult, op1=ALU.add)
            nc.vector.tensor_tensor(invp, invp, t_b, op=ALU.mult)
            ps_qb = psaux.tile([128, 1], F32, tag="aux")
            nc.tensor.matmul(ps_qb, ones_row, invp, start=True, stop=True)
            qhat = cp.tile([128, 1], F32)
            nc.vector.tensor_tensor(qhat, qp, ps_qb, op=ALU.mult)

            # ---- prelude: token budget theta = min(512 + 256*sigmoid, cv) - 0.5
            pooled = cp.tile([128, KCH], F32)
            for k in range(KCH):
                ps_pk = psaux.tile([128, 1], F32, tag="aux")
                nc.tensor.matmul(ps_pk, qn_sb[:, k * 128:(k + 1) * 128], ones64,
                                 start=True, stop=True)
                nc.scalar.activation(pooled[:, k:k + 1], ps_pk, AF.Copy,
                                     scale=1.0 / Q)
            ps_h = psaux.tile([128, 1], F32, tag="aux")
            for k in range(KCH):
                nc.tensor.matmul(ps_h, w1_sb[:, k, :], pooled[:, k:k + 1],
                                 start=(k == 0), stop=(k == KCH - 1))
            hidden = cp.tile([128, 1], F32)
            nc.scalar.activation(hidden, ps_h, AF.Relu, bias=b1_ap)
            ps_c = psaux.tile([1, 1], F32, tag="aux")
            nc.tensor.matmul(ps_c, w2_ap, hidden, start=True, stop=True)
            vbud = cp.tile([1, 1], F32)
            nc.scalar.activation(vbud, ps_c, AF.Sigmoid, bias=b2_ap)
            nc.vector.tensor_scalar(vbud, vbud, 256.0, 512.0,
                                    op0=ALU.mult, op1=ALU.add)
            maskf = cp.tile([128, NCOL], F32)
            nc.vector.tensor_copy(maskf, mask_sb)
            cv_p = cp.tile([128, 1], F32)
            nc.vector.reduce_sum(cv_p, maskf, axis=mybir.AxisListType.X)
            ps_cv = psaux.tile([1, 1], F32, tag="aux")
            nc.tensor.matmul(ps_cv, ones128, cv_p, start=True, stop=True)
            theta = cp.tile([1, 1], F32)
            nc.vector.tensor_tensor(theta, vbud, ps_cv, op=ALU.min)
            nc.vector.tensor_scalar(theta, theta, 0.5, None, op0=ALU.subtract)

            # ---- main loop: stream contexts, project, u / n2 ----
            u_all = cp.tile([128, NCOL], F32)
            n2_all = cp.tile([128, NCOL], F32)

            def emit_nu(t, z_sb, z2_sb):
                for c in range(T // 128):
                    col = t * (T // 128) + c
                    ps_u = psnu.tile([128, 1], F32, tag="pnu")
                    nc.tensor.matmul(ps_u, z_sb[:, c * 128:(c + 1) * 128], qhat,
                                     start=True, stop=True)
                    nc.scalar.activation(u_all[:, col:col + 1], ps_u, AF.Copy)
                    ps_n2 = psnu.tile([128, 1], F32, tag="pnu")
                    nc.tensor.matmul(ps_n2, z2_sb[:, c * 128:(c + 1) * 128],
                                     ones128, start=True, stop=True)
                    nc.scalar.activation(n2_all[:, col:col + 1], ps_n2, AF.Copy)

            zhist = {}
            xh2 = xl2 = None
            for t in range(NT):
                if t % 2 == 0:
                    xh2 = xp.tile([128, KCH, 2 * T], F16, tag="xh")
                    nc.sync.dma_start(out=xh2, in_=xhi[:, :, t * T:(t + 2) * T])
                    xl2 = xp.tile([128, KCH, 2 * T], F16, tag="xl")
                    nc.sync.dma_start(out=xl2, in_=xlo[:, :, t * T:(t + 2) * T])
                half = (t % 2) * T
                xh = xh2[:, :, half:half + T]
                xl = xl2[:, :, half:half + T]
                ps_z = psz.tile([128, T], F32, tag="pz")
                nmm = 3 * KCH
                i = 0
                for k in range(KCH):
                    for (w_sb, x_sb) in ((wch_sb, xh), (wcl_sb, xh), (wch_sb, xl)):
                        nc.tensor.matmul(ps_z, w_sb[:, k, :], x_sb[:, k, :],
                                         start=(i == 0), stop=(i == nmm - 1))
                        i += 1
                z_sb = zp.tile([128, T], F32, tag="z")
                nc.scalar.activation(z_sb, ps_z, AF.Identity, bias=bc_ap)
                z2_sb = zp.tile([128, T], F32, tag="z2")
                nc.vector.tensor_tensor(z2_sb, z_sb, z_sb, op=ALU.mult)
                zhist[t] = (z_sb, z2_sb)
                if t >= 1:
                    emit_nu(t - 1, *zhist.pop(t - 1))
            emit_nu(NT - 1, *zhist.pop(NT - 1))

            # ---- scores = u * rsqrt(n2), masked to -inf ----
            sq_all = cp.tile([128, NCOL], F32)
            nc.scalar.activation(sq_all, n2_all, AF.Sqrt, bias=zero_col)
            inv_all = cp.tile([128, NCOL], F32)
            nc.vector.reciprocal(inv_all, sq_all)
            t_c = cp.tile([128, NCOL], F32, tag="nt_c")
            nc.vector.tensor_tensor(t_c, inv_all, inv_all, op=ALU.mult)
            nc.vector.tensor_tensor(t_c, t_c, n2_all, op=ALU.mult)
            nc.vector.tensor_scalar(t_c, t_c, -0.5, 1.5, op0=ALU.mult, op1=ALU.add)
            nc.vector.tensor_tensor(inv_all, inv_all, t_c, op=ALU.mult)
            scores = cp.tile([128, NCOL], F32)
            nc.vector.tensor_tensor(scores, u_all, inv_all, op=ALU.mult)
            ninf = cp.tile([128, NCOL], F32)
            nc.vector.memset(ninf, float("-inf"))
            smask = cp.tile([128, NCOL], F32)
            nc.vector.select(smask, mask_sb, scores, ninf)
            nc.sync.dma_start(out=scores_out[:], in_=smask)

            # ---- bisection for the budget-th largest score ----
            lo = cp.tile([1, 1], F32)
            nc.vector.memset(lo, -1.024)
            mid = cp.tile([1, 1], F32)
            cond = cp.tile([1, 1], F32)
            ge = cp.tile([128, NCOL], F32)
            cnt_p = cp.tile([128, 1], F32)
            for i in range(bisect_iters):
                half = 1.024 / (2.0 ** i)
                nc.vector.tensor_scalar(mid, lo, half, None, op0=ALU.add)
                ps_mb = psnu.tile([128, 1], F32, tag="pnu")
                nc.tensor.matmul(ps_mb, ones_row, mid, start=True, stop=True)
                nc.vector.tensor_scalar(ge, smask, ps_mb, None,
                                        op0=ALU.is_ge, op1=ALU.add,
                                        accum_out=cnt_p)
                ps_ct = psaux.tile([1, 1], F32, tag="aux")
                nc.tensor.matmul(ps_ct, ones128, cnt_p, start=True, stop=True)
                nc.vector.tensor_scalar(cond, ps_ct, theta, None, op0=ALU.is_ge)
                nc.vector.scalar_tensor_tensor(lo, cond, half, lo,
                                               op0=ALU.mult, op1=ALU.add)

            ps_lb = psnu.tile([128, 1], F32, tag="pnu")
            nc.tensor.matmul(ps_lb, ones_row, lo, start=True, stop=True)
            selm = cp.tile([128, NCOL], F32)
            nc.vector.tensor_scalar(selm, smask, ps_lb, None, op0=ALU.is_ge)
            nc.sync.dma_start(out=mask_out[:], in_=selm)

    bass_rust.generate_event_semaphores(nc)
    return nc


def _prep_core_inputs(inputs, b, C=C):
    NCOL = C // 128
    ctx = np.asarray(inputs["context_embeddings"][b], np.float32)     # [C, E]
    qe = np.asarray(inputs["query_embeddings"][b], np.float32)        # [Q, E]
    cm = np.asarray(inputs["context_mask"][b])                        # [C]

    xT = np.ascontiguousarray(
        ctx.T.reshape(KCH, 128, C).transpose(1, 0, 2))                # [128,KCH,C]
    xhi = xT.astype(np.float16)
    xlo = (xT - xhi.astype(np.float32)).astype(np.float16)
    q_embT = np.ascontiguousarray(
        qe.T.reshape(KCH, 128, Q).transpose(1, 0, 2))                 # [128,KCH,Q]

    def wdev(w):
        return np.ascontiguousarray(
            np.asarray(w, np.float32).reshape(KCH, 128, -1).transpose(1, 0, 2))

    smalls = np.zeros((128, 8), np.float32)
    smalls[:, 0] = np.asarray(inputs["W2"], np.float32)[:, 0]
    smalls[:, 1] = np.asarray(inputs["bq"], np.float32)
    smalls[:, 2] = np.asarray(inputs["bc"], np.float32)
    smalls[:, 3] = np.asarray(inputs["b1"], np.float32)
    smalls[0, 4] = np.float32(np.asarray(inputs["b2"], np.float32)[0])

    mask_u8 = np.ascontiguousarray(
        cm.astype(np.uint8).reshape(NCOL, 128).T)                     # [128,NCOL]

    wc32 = wdev(inputs["Wc"])
    wch = wc32.astype(np.float16)
    wcl = (wc32 - wch.astype(np.float32)).astype(np.float16)
    return {
        "xhi": xhi,
        "xlo": xlo,
        "wch": wch,
        "wcl": wcl,
        "q_nat": np.ascontiguousarray(qe),
        "q_embT": q_embT,
        "wq": wdev(inputs["Wq"]),
        "w1": wdev(inputs["W1"]),
        "smalls": smalls,
        "mask_in": mask_u8,
    }


def kernel(**inputs):
    if "nc" not in _NC_CACHE:
        _NC_CACHE["nc"] = build_nc()
    nc = _NC_CACHE["nc"]

    in_maps = [_prep_core_inputs(inputs, b) for b in range(B)]
    res = run_bass_kernel_spmd(nc, in_maps, core_ids=list(range(B)))

    selection_mask = np.empty((B, C), np.float32)
    scores = np.empty((B, C), np.float32)
    for b in range(B):
        selection_mask[b] = res.results[b]["mask_out"].T.reshape(-1)
        scores[b] = res.results[b]["scores_out"].T.reshape(-1)
    return selection_mask, scores
